# revision 7
# baseline (speedup 1.0000x reference)
"""AttnBlock (GroupNorm + 1x1-conv QKV + spatial attention w/ softmax over
query-h + out-proj + residual) for Trainium2, 8 NeuronCores.

Sharding: core = 2*b + w_half  (4 samples x 2 halves of the w axis).
The softmax normalizes over the h index of the *query*, so for a fixed w
column the 64 h-values form one softmax group; splitting by w keeps every
group on one core.

v2 design (vs the fp32r baseline):
  - Query packing is h-major inside each 512-query block: q = qt*512 + h*8
    + ww (ww = w' - 8*qt). The softmax-normalize multiply then has its
    broadcast on a *middle* axis and packed bf16 innermost, which unlocks
    the DVE 2x_1p fast mode.
  - S = K^T Q runs as ONE fp8e4 DoubleRow matmul per (key-block, q-block):
    both operands are [ki, 2, *] with channel c = t*128 + ki. 2x PE rate.
  - exp reads a 2-bank [128, 1024] PSUM span in one ACT op, writes bf16.
  - d (softmax denominators, per (key, w')) via GpSimd half-fold (bf16
    tensor_add) + DVE strided segmented reduce; reciprocal on DVE (bf16).
  - attn = e * r broadcast-multiply on DVE at 2x (a few pairs on GpSimd).
  - O = V^T attn accumulates in bf16 (V^T tiles cast to bf16 at conv time).
  - GroupNorm is folded into the conv weights on device (as baseline).
"""

import os

import numpy as np

import concourse.bass as bass
import concourse.bacc as bacc
import concourse.mybir as mybir
import concourse.tile as tile
from concourse.bass_utils import run_bass_kernel_spmd

B, C, H, W = 4, 256, 64, 64
N = H * W            # 4096 keys
NH = N // 2          # 2048 queries per w-half
WH = W // 2          # 32 local w' values
GROUPS = 32
EPS = 1e-5
F32 = mybir.dt.float32
F32R = mybir.dt.float32r
BF16 = mybir.dt.bfloat16
FP8 = mybir.dt.float8e4
AF = mybir.ActivationFunctionType
ALU = mybir.AluOpType
AX = mybir.AxisListType
DR = mybir.MatmulPerfMode.DoubleRow


def _r(ap):
    return ap.bitcast(F32R)


def _bcast_mid(ap, n):
    """[p, ..., m] AP -> [p, ..., 0 x n, m]: broadcast over a new middle
    axis, keeping the packed innermost dim (preserves DVE 2x_1p)."""
    return bass.AP(tensor=ap.tensor, offset=ap.offset,
                   ap=[*ap.ap[:-1], [0, n], ap.ap[-1]])


def build_nc():
    nc = bacc.Bacc("TRN2", target_bir_lowering=False, debug=False)

    xf_d = nc.dram_tensor("xf", [C, N], F32, kind="ExternalInput")
    xh_d = nc.dram_tensor("xh", [C, NH], F32, kind="ExternalInput")
    wT_d = {t: nc.dram_tensor(f"w{t}T", [C, C], F32, kind="ExternalInput")
            for t in "qkvo"}
    brow_d = {"v": nc.dram_tensor("bv_row", [1, C], F32, kind="ExternalInput")}
    bcol_d = {t: nc.dram_tensor(f"b{t}_col", [C, 1], F32, kind="ExternalInput")
              for t in "qko"}
    gamma_d = nc.dram_tensor("gamma_c", [C, 1], F32, kind="ExternalInput")
    beta_d = nc.dram_tensor("beta_c", [C, 1], F32, kind="ExternalInput")
    g1_d = nc.dram_tensor("G1", [C, GROUPS], F32, kind="ExternalInput")
    g2_d = nc.dram_tensor("G2", [GROUPS, C], F32, kind="ExternalInput")
    ones_d = nc.dram_tensor("ones_row", [1, 512], F32, kind="ExternalInput")
    out_d = nc.dram_tensor("out", [C, NH], F32, kind="ExternalOutput")

    with tile.TileContext(nc) as tc:
        with (
            nc.allow_low_precision(reason="bf16 softmax pipeline, 2e-2 gate"),
            tc.tile_pool(name="persist", bufs=1) as pp,
            tc.tile_pool(name="mm", bufs=3, space="PSUM") as pmm,
            tc.tile_pool(name="opsum", bufs=2, space="PSUM") as pop,
            tc.tile_pool(name="epool", bufs=4) as pe_pool,
            tc.tile_pool(name="upool", bufs=4) as pu_pool,
            tc.tile_pool(name="dpool", bufs=6) as pd_pool,
            tc.tile_pool(name="outpool", bufs=3) as pout,
        ):
            def ptile(shape, tag, dtype=F32):
                return pp.tile(shape, dtype, tag=tag, name=tag)

            def psum_t(tag_name):
                # [128, 1024] fp32 = 2 PSUM banks
                return pmm.tile([128, 1024], F32, tag="mm", name=tag_name)

            # ---------------- loads ----------------
            xf = []
            xh = []
            wT = {t: [] for t in "qkvo"}
            gam, bet, g1 = [], [], []
            for i in range(2):
                t = ptile([128, N], f"xf{i}", F32R)
                for ch in range(4):
                    nc.sync.dma_start(
                        out=t[:, 1024 * ch:1024 * (ch + 1)],
                        in_=xf_d[128 * i:128 * (i + 1),
                                 1024 * ch:1024 * (ch + 1)].bitcast(F32R))
                xf.append(t)
                t = ptile([128, NH], f"xh{i}", F32R)
                for ch in range(2):
                    nc.sync.dma_start(
                        out=t[:, 1024 * ch:1024 * (ch + 1)],
                        in_=xh_d[128 * i:128 * (i + 1),
                                 1024 * ch:1024 * (ch + 1)].bitcast(F32R))
                xh.append(t)
                for w in "qkvo":
                    t = ptile([128, C], f"w{w}T{i}", F32R)
                    nc.sync.dma_start(out=t, in_=wT_d[w][128 * i:128 * (i + 1), :].bitcast(F32R))
                    wT[w].append(t)
                t = ptile([128, 1], f"gam{i}")
                nc.sync.dma_start(out=t, in_=gamma_d[128 * i:128 * (i + 1), :])
                gam.append(t)
                t = ptile([128, 1], f"bet{i}")
                nc.sync.dma_start(out=t, in_=beta_d[128 * i:128 * (i + 1), :])
                bet.append(t)
                t = ptile([128, GROUPS], f"g1_{i}")
                nc.sync.dma_start(out=t, in_=g1_d[128 * i:128 * (i + 1), :])
                g1.append(t)
            g2 = ptile([GROUPS, C], "g2")
            nc.sync.dma_start(out=g2, in_=g2_d[:, :])
            ones = ptile([1, 512], "ones", F32R)
            nc.sync.dma_start(out=ones, in_=ones_d[:, :].bitcast(F32R))
            brow = {}
            for w in "v":
                brow[w] = ptile([1, C], f"b{w}row", F32R)
                nc.sync.dma_start(out=brow[w], in_=brow_d[w][:, :].bitcast(F32R))
            bcol = {}
            for w in "qko":
                bcol[w] = []
                for i in range(2):
                    t = ptile([128, 1], f"b{w}col{i}")
                    nc.sync.dma_start(out=t, in_=bcol_d[w][128 * i:128 * (i + 1), :])
                    bcol[w].append(t)

            # ---------------- GroupNorm stats -> per-channel scale/shift ----
            NSUB = N // 512
            mstat = []
            for i in range(2):
                stats = pd_pool.tile([128, NSUB, 6], F32, tag="gnstats",
                                     name=f"gnstats{i}")
                for s in range(NSUB):
                    nc.vector.bn_stats(out=stats[:, s, :],
                                       in_=xf[i][:, 512 * s:512 * (s + 1)].bitcast(F32))
                mv = pd_pool.tile([128, 2], F32, tag="gnmv", name=f"gnmv{i}")
                nc.vector.bn_aggr(out=mv, in_=stats)
                ms = ptile([128, 2], f"mstat{i}")
                # ms = [mean_c, E[x^2]_c]
                nc.vector.tensor_mul(out=ms[:, 1:2], in0=mv[:, 0:1], in1=mv[:, 0:1])
                nc.vector.tensor_add(out=ms[:, 1:2], in0=ms[:, 1:2], in1=mv[:, 1:2])
                nc.vector.tensor_copy(out=ms[:, 0:1], in_=mv[:, 0:1])
                mstat.append(ms)

            pg_t = psum_t("pg")
            pg = pg_t[:GROUPS, :2]
            for i in range(2):
                nc.tensor.matmul(pg, lhsT=g1[i], rhs=mstat[i],
                                 start=(i == 0), stop=(i == 1))
            gstat = ptile([GROUPS, 2], "gstat")
            nc.vector.tensor_scalar_mul(out=gstat, in0=pg, scalar1=1.0 / 8.0)
            var32 = ptile([GROUPS, 1], "var32")
            nc.vector.tensor_mul(out=var32, in0=gstat[:, 0:1], in1=gstat[:, 0:1])
            nc.vector.tensor_sub(out=var32, in0=gstat[:, 1:2], in1=var32)
            std32 = ptile([GROUPS, 1], "std32")
            eps_t = ptile([GROUPS, 1], "eps_t")
            nc.vector.memset(eps_t, EPS)
            nc.scalar.activation(out=std32, in_=var32, func=AF.Sqrt, bias=eps_t)
            rstd = ptile([GROUPS, 1], "rstd")
            nc.vector.reciprocal(out=rstd, in_=std32)
            # one Newton polish of rsqrt: y <- y*(1.5 - 0.5*(var+eps)*y^2)
            tnr = ptile([GROUPS, 1], "tnr")
            nc.vector.tensor_mul(out=tnr, in0=rstd, in1=rstd)
            nc.vector.tensor_mul(out=tnr, in0=tnr, in1=var32)
            vepsy = ptile([GROUPS, 1], "vepsy")
            nc.vector.tensor_mul(out=vepsy, in0=rstd, in1=rstd)
            nc.vector.tensor_scalar_mul(out=vepsy, in0=vepsy, scalar1=EPS)
            nc.vector.tensor_add(out=tnr, in0=tnr, in1=vepsy)
            nc.vector.tensor_scalar_mul(out=tnr, in0=tnr, scalar1=-0.5)
            nc.vector.tensor_scalar_add(out=tnr, in0=tnr, scalar1=1.5)
            nc.vector.tensor_mul(out=rstd, in0=rstd, in1=tnr)

            grstat = ptile([GROUPS, 2], "grstat")
            nc.vector.tensor_copy(out=grstat[:, 0:1], in_=gstat[:, 0:1])
            nc.vector.tensor_copy(out=grstat[:, 1:2], in_=rstd)

            sc, sh = [], []
            for i in range(2):
                pc_t = psum_t(f"pc{i}")
                pc = pc_t[:128, :2]
                nc.tensor.matmul(pc, lhsT=g2[:, 128 * i:128 * (i + 1)],
                                 rhs=grstat, start=True, stop=True)
                s = ptile([128, 1], f"sc{i}")
                nc.vector.tensor_mul(out=s, in0=pc[:, 1:2], in1=gam[i])
                sc.append(s)
                h = ptile([128, 1], f"sh{i}", F32R)
                nc.vector.tensor_mul(out=h, in0=pc[:, 0:1], in1=s)
                nc.vector.tensor_sub(out=h, in0=bet[i], in1=h)
                sh.append(h)

            # effective v bias as a row (per-free-column bias for V^T)
            beffr = {}
            for w in "v":
                rp_t = psum_t(f"br{w}")
                rp = rp_t[:1, :C]
                for i in range(2):
                    nc.tensor.matmul(rp, lhsT=sh[i], rhs=wT[w][i],
                                     start=(i == 0), stop=(i == 1))
                bt = ptile([1, C], f"beff{w}", F32R)
                nc.vector.tensor_add(out=bt, in0=rp, in1=brow[w])
                beffr[w] = bt
            # effective q,k biases as columns (per-partition bias for ACT fuse)
            beffc = {}
            for w in "qk":
                beffc[w] = []
                for j in range(2):
                    bp_t = psum_t(f"bc{w}{j}")
                    bp = bp_t[:128, :1]
                    for i in range(2):
                        nc.tensor.matmul(bp,
                                         lhsT=wT[w][i][:, 128 * j:128 * (j + 1)].bitcast(F32),
                                         rhs=sh[i].bitcast(F32),
                                         start=(i == 0), stop=(i == 1))
                    t = ptile([128, 1], f"beffc{w}{j}")
                    nc.vector.tensor_add(out=t, in0=bp, in1=bcol[w][j])
                    beffc[w].append(t)

            # scale conv weights in place: WeffT[i,o] = wT[i,o] * scale_i
            for w in "qkv":
                for i in range(2):
                    nc.vector.tensor_scalar_mul(out=wT[w][i], in0=wT[w][i],
                                                scalar1=sc[i])

            # ---------------- convs: K, Q (fp8 DoubleRow layout), V^T (bf16)
            # k8/q8 layout [ki, t, col]: channel c = t*128 + ki
            k8 = ptile([128, 2, N], "k8", FP8)
            q8 = ptile([128, 2, NH], "q8", FP8)
            for j in range(2):
                for s2 in range(N // 1024):
                    kp = psum_t(f"kp{j}_{s2}")
                    for half in range(2):
                        cols = slice(1024 * s2 + 512 * half,
                                     1024 * s2 + 512 * (half + 1))
                        for i in range(2):
                            nc.tensor.matmul(
                                kp[:, 512 * half:512 * (half + 1)],
                                lhsT=_r(wT["k"][i][:, 128 * j:128 * (j + 1)]),
                                rhs=_r(xf[i][:, cols]),
                                start=(i == 0), stop=(i == 1))
                    nc.scalar.activation(
                        out=k8[:, j, 1024 * s2:1024 * (s2 + 1)],
                        in_=kp, func=AF.Identity, bias=beffc["k"][j])
            for j in range(2):
                for s2 in range(NH // 1024):
                    qp = psum_t(f"qp{j}_{s2}")
                    for half in range(2):
                        cols = slice(1024 * s2 + 512 * half,
                                     1024 * s2 + 512 * (half + 1))
                        for i in range(2):
                            nc.tensor.matmul(
                                qp[:, 512 * half:512 * (half + 1)],
                                lhsT=_r(wT["q"][i][:, 128 * j:128 * (j + 1)]),
                                rhs=_r(xh[i][:, cols]),
                                start=(i == 0), stop=(i == 1))
                    nc.scalar.activation(
                        out=q8[:, j, 1024 * s2:1024 * (s2 + 1)],
                        in_=qp, func=AF.Identity, bias=beffc["q"][j])

            vT = []
            for rt in range(N // 128):
                vp_t = psum_t(f"vp{rt}")
                vp = vp_t[:, :C]
                for i in range(2):
                    nc.tensor.matmul(vp,
                                     lhsT=_r(xf[i][:, 128 * rt:128 * (rt + 1)]),
                                     rhs=_r(wT["v"][i]),
                                     start=(i == 0), stop=False)
                nc.tensor.matmul(vp, lhsT=_r(ones[:, :128]), rhs=_r(beffr["v"]),
                                 start=False, stop=True)
                t = ptile([128, C], f"vT{rt}", BF16)
                eng = nc.vector if rt % 2 == 0 else nc.scalar
                if eng is nc.scalar:
                    nc.scalar.activation(out=t, in_=vp, func=AF.Identity)
                else:
                    nc.vector.tensor_copy(out=t, in_=vp)
                vT.append(t)

            # bf16 copy of out-proj weights (O path runs bf16)
            wo_bf = []
            for i in range(2):
                t = ptile([128, C], f"wo_bf{i}", BF16)
                nc.vector.tensor_copy(out=t, in_=wT["o"][i].bitcast(F32))
                wo_bf.append(t)

            o_sb = [ptile([128, NH], "o_sb0", BF16), ptile([128, NH], "o_sb1", BF16)]

            kstage = os.environ.get("KSTAGE", "full")
            if kstage == "conv":
                for ct in range(2):
                    dbg = pout.tile([128, NH], F32, tag="dbg", name=f"dbg{ct}",
                                    bufs=2)
                    nc.vector.tensor_copy(out=dbg, in_=k8[:, ct, :NH])
                    nc.sync.dma_start(out=out_d[128 * ct:128 * (ct + 1), :],
                                      in_=dbg)

            # ---------------- attention ----------------
            NPAIR = N // 256     # 16 pairs of 128-key blocks
            for qt in range(4 if kstage != "conv" else 0):
                o_ps = [pop.tile([128, 512], F32, tag="o", name=f"ops{qt}_{ct}")
                        for ct in range(2)]
                qcols = slice(512 * qt, 512 * (qt + 1))
                d_t = None
                for p in range(NPAIR):
                    gidx = qt * NPAIR + p
                    sp = psum_t(f"sp{qt}_{p}")
                    for t01 in range(2):
                        kb = 256 * p + 128 * t01
                        nc.tensor.matmul(
                            sp[:, 512 * t01:512 * (t01 + 1)],
                            lhsT=k8[:, :, kb:kb + 128],
                            rhs=q8[:, :, qcols],
                            start=True, stop=True, perf_mode=DR)
                    # exp(S/16) -> bf16, one 2-bank ACT op
                    e_t = pe_pool.tile([128, 1024], BF16, tag="e",
                                       name=f"e{qt}_{p}")
                    nc.scalar.activation(out=e_t, in_=sp, func=AF.Exp,
                                         scale=1.0 / 16.0)
                    # view [p, t, h, ww]
                    e4 = e_t.rearrange("p (t h w) -> p t h w", t=2, w=8)
                    if kstage != "attn_nonorm":
                        # GpSimd half-fold over h, then DVE segmented reduce
                        u_t = pu_pool.tile([128, 512], BF16, tag="u",
                                           name=f"u{qt}_{p}")
                        u4 = u_t.rearrange("p (t h w) -> p t h w", t=2, w=8)
                        nc.gpsimd.tensor_add(out=u4, in0=e4[:, :, 0:32, :],
                                             in1=e4[:, :, 32:64, :])
                        d_t = pd_pool.tile([128, 16], F32, tag="d",
                                           name=f"d{qt}_{p}")
                        r_t = pd_pool.tile([128, 16], BF16, tag="r",
                                           name=f"r{qt}_{p}")
                        # reduce over folded h (stride-8 innermost view)
                        nc.vector.tensor_reduce(
                            out=d_t.rearrange("p (t w) -> p t w", t=2),
                            in_=u_t.rearrange("p (t h w) -> p t w h", t=2, w=8),
                            axis=AX.X, op=ALU.add)
                        nc.vector.reciprocal(out=r_t, in_=d_t)
                        # attn = e * r (broadcast over h via middle axis; 2x_1p)
                        r_b = bass.AP(tensor=r_t.tensor, offset=r_t.offset,
                                      ap=[r_t.ap[0], [8, 2], [0, 64], [1, 8]])
                        mul_eng = (nc.gpsimd
                                   if gidx % 16 in (5, 10, 15)
                                   and kstage != "attn_dve" else nc.vector)
                        mul_eng.tensor_mul(out=e4, in0=e4, in1=r_b)
                    for ct in range(2):
                        for t01 in range(2):
                            nc.tensor.matmul(
                                o_ps[ct],
                                lhsT=vT[2 * p + t01][:, 128 * ct:128 * (ct + 1)],
                                rhs=e_t[:, 512 * t01:512 * (t01 + 1)],
                                start=(p == 0 and t01 == 0),
                                stop=(p == NPAIR - 1 and t01 == 1))
                for ct in range(2):
                    nc.scalar.activation(out=o_sb[ct][:, qcols], in_=o_ps[ct],
                                         func=AF.Identity)

                # ---------------- out-proj + residual for this quarter -----
                for ct in range(2):
                    prj_t = psum_t(f"prj{qt}_{ct}")
                    prj = prj_t[:, :512]
                    for i in range(2):
                        nc.tensor.matmul(
                            prj,
                            lhsT=wo_bf[i][:, 128 * ct:128 * (ct + 1)],
                            rhs=o_sb[i][:, qcols],
                            start=(i == 0), stop=(i == 1))
                    ot = pout.tile([128, 512], F32, tag="ot",
                                   name=f"ot{qt}_{ct}")
                    nc.vector.scalar_tensor_tensor(
                        out=ot, in0=prj, scalar=bcol["o"][ct],
                        in1=xh[ct][:, qcols].bitcast(F32),
                        op0=ALU.add, op1=ALU.add)
                    nc.sync.dma_start(out=out_d[128 * ct:128 * (ct + 1), qcols],
                                      in_=ot)
    nc.compile()
    return nc


_NC = None


def _get_nc():
    global _NC
    if _NC is None:
        _NC = build_nc()
    return _NC


def _prep_in_maps(x, gamma, beta, q_w, q_b, k_w, k_b, v_w, v_b, o_w, o_b):
    x = np.ascontiguousarray(np.asarray(x, np.float32))
    g1 = np.zeros((C, GROUPS), np.float32)
    g1[np.arange(C), np.arange(C) // (C // GROUPS)] = 1.0
    shared = {
        "gamma_c": np.asarray(gamma, np.float32).reshape(C, 1).copy(),
        "beta_c": np.asarray(beta, np.float32).reshape(C, 1).copy(),
        "G1": g1,
        "G2": np.ascontiguousarray(g1.T),
        "ones_row": np.ones((1, 512), np.float32),
    }
    for t, wm, bv in (("q", q_w, q_b), ("k", k_w, k_b),
                      ("v", v_w, v_b), ("o", o_w, o_b)):
        shared[f"w{t}T"] = np.ascontiguousarray(np.asarray(wm, np.float32).T)
        if t == "v":
            shared["bv_row"] = np.asarray(bv, np.float32).reshape(1, C).copy()
        else:
            shared[f"b{t}_col"] = np.asarray(bv, np.float32).reshape(C, 1).copy()
    in_maps = []
    for core in range(8):
        b, half = core // 2, core % 2
        xb = x[b].reshape(C, N)
        # queries h-major inside each 512 block: q = qt*512 + h*8 + ww
        xh = x[b][:, :, half * WH:(half + 1) * WH]           # [C, 64h, 32w']
        xh = np.ascontiguousarray(
            xh.reshape(C, H, 4, 8).transpose(0, 2, 1, 3)
        ).reshape(C, NH)
        in_maps.append(dict(shared, xf=np.ascontiguousarray(xb), xh=xh))
    return in_maps


def run(trace=False, **inputs):
    in_maps = _prep_in_maps(**inputs)
    nc = _get_nc()
    res = run_bass_kernel_spmd(nc, in_maps, core_ids=list(range(8)), trace=trace)
    x = np.asarray(inputs["x"], np.float32)
    out = np.empty((B, C, H, W), np.float32)
    for core in range(8):
        b, half = core // 2, core % 2
        od = res.results[core]["out"]                        # [C, 2048]
        oh = od.reshape(C, 4, H, 8).transpose(0, 2, 1, 3).reshape(C, H, WH)
        out[b][:, :, half * WH:(half + 1) * WH] = oh
    return out, res


def kernel(**inputs):
    out, _ = run(trace=False, **inputs)
    return out
